# revision 1
# baseline (speedup 1.0000x reference)
"""Trainium2 Bass kernel: MultiHeadLatentAttention.

Problem (hardcoded): B=4, S=1024, HID=2048, NH=16 heads of HD=128, LAT=512,
fp32, causal attention with RoPE, latent-compressed K/V (MLA).

Sharding over 8 NeuronCores: core c = (batch b = c//2, head-group hg = c%2).
Each core handles one batch element and 8 heads (local width HL=1024).

Device-side layout strategy (everything transposed so the contraction dim
always sits on SBUF partitions):
  xT      [HID, S]   (host passes x[b].T)
  QT = (x Wq + bq).T          -> [HL, S]  per head h: QT[h*128:(h+1)*128] = q_h.T
  latT = (x Wdown).T          -> [LAT, S]
  KT = (lat Wk_up).T          -> [HL, S]
  V  = lat Wv_up  (natural)   -> [S, HL]
  RoPE on QT/KT: q*cos + rotate_half(q)*sin, computed as
      qT*cosT + shift64(qT)*sinTe   (sign of sin folded into sinTe by host)
  scoresT_h = k_h @ q_h.T     -> [k, q] blocks   (lhsT = KT block, rhs = QT)
  expT = exp(scoresT/sqrt(128)); diagonal blocks are column-sliced to the
      unmasked range and the residual triangle is zeroed with a binary mask
  sums[1, q]  = ones.T @ expT  (PE reduction over k partitions)
  ctxT_h[d,q] = v_h.T @ ... accumulated:  matmul(lhsT=V block, rhs=expT)
  normalize: bcast = ones_col.T @ sums (K=1 matmul partition-broadcast),
      ctxT *= 1/bcast
  outT_partial = Wo_s.T @ ctxT -> [HID, S], DMA'd from PSUM to DRAM.

Host gathers: out[b] = (outT[2b] + outT[2b+1]).T + bo.
"""

import os

if "axon" not in os.environ.get("JAX_PLATFORMS", ""):
    os.environ["JAX_PLATFORMS"] = "axon"

import numpy as np

import concourse.bacc as bacc
import concourse.mybir as mybir
import concourse.tile as tile
from concourse.bass_utils import run_bass_kernel_spmd

# ---- problem dims (hardcoded per contest rules)
B, S, HID, NH, LAT = 4, 1024, 2048, 16, 512
HD = 128
NHL = NH // 2          # heads per core = 8
HL = NHL * HD          # local head width = 1024
P = 128
KT_H = HID // P        # 16
KT_L = LAT // P        # 4
QCW = 512              # q-chunk width (fp32 matmul moving limit / PSUM bank)
NQC = S // QCW         # 2
SC_SCALE = float(1.0 / np.sqrt(HD))

F32 = mybir.dt.float32
F32R = mybir.dt.float32r

N_CORES = 8


def _rope(nc, pool, raw, out_ap, cosT_sb, sinTe_sb, dve_sin=False,
          dma_shift=False):
    """out = raw * cosT + shift64(raw) * sinTe on a full [128, S] tile."""
    sh = pool.tile([P, S], F32, tag="shift", name="sh")
    if dma_shift:  # use DMA when the HBM queues are idle in this phase
        nc.sync.dma_start(sh[0:64, :], raw[64:128, :])
        nc.sync.dma_start(sh[64:128, :], raw[0:64, :])
    else:
        nc.gpsimd.tensor_copy(sh[0:64, :], raw[64:128, :])
        nc.gpsimd.tensor_copy(sh[64:128, :], raw[0:64, :])
    nc.vector.tensor_mul(out_ap, raw, cosT_sb)
    if dve_sin:
        nc.vector.tensor_mul(sh, sh, sinTe_sb)
    else:
        nc.gpsimd.tensor_mul(sh, sh, sinTe_sb)
    nc.vector.tensor_add(out_ap, out_ap, sh)


def build_bass(loop_iters=None):
    nc = bacc.Bacc("TRN2", target_bir_lowering=False, debug=False, num_devices=8)

    xT = nc.dram_tensor("xT", [HID, S], F32, kind="ExternalInput")[:]
    wq = nc.dram_tensor("wq", [HID, HL], F32, kind="ExternalInput")[:]
    wdown = nc.dram_tensor("wdown", [HID, LAT], F32, kind="ExternalInput")[:]
    wkup = nc.dram_tensor("wkup", [LAT, HL], F32, kind="ExternalInput")[:]
    wvup = nc.dram_tensor("wvup", [LAT, HL], F32, kind="ExternalInput")[:]
    wo = nc.dram_tensor("wo", [HL, HID], F32, kind="ExternalInput")[:]
    bqd = nc.dram_tensor("bq", [P, NHL], F32, kind="ExternalInput")[:]
    cosTd = nc.dram_tensor("cosT", [P, S], F32, kind="ExternalInput")[:]
    sinTed = nc.dram_tensor("sinTe", [P, S], F32, kind="ExternalInput")[:]
    maskTd = nc.dram_tensor("maskT", [P, 3 * P], F32, kind="ExternalInput")[:]
    onescd = nc.dram_tensor("ones_c", [1, P], F32, kind="ExternalInput")[:]
    oneskd = nc.dram_tensor("ones_r", [P, 1], F32, kind="ExternalInput")[:]
    outT = nc.dram_tensor("outT", [HID, S], F32, kind="ExternalOutput")[:]

    import contextlib

    with tile.TileContext(nc) as tc, contextlib.ExitStack() as _les:
        if loop_iters is not None:
            _les.enter_context(tc.For_i(0, loop_iters, 1))
        with (
            tc.tile_pool(name="consts", bufs=1) as consts,
            tc.tile_pool(name="resident", bufs=1) as resident,
            tc.tile_pool(name="psc", bufs=2, space="PSUM") as psc,
        ):
            cosT_sb = consts.tile([P, S], F32)
            sinTe_sb = consts.tile([P, S], F32)
            mask_sb = consts.tile([P, 3 * P], F32)
            bq_sb = consts.tile([P, NHL], F32)
            ones_col = consts.tile([1, P], F32R)
            ones_k = consts.tile([P, 1], F32R)

            latT_sb = resident.tile([P, KT_L, S], F32R)
            qT_sb = resident.tile([P, NHL, S], F32R)
            kT_sb = resident.tile([P, NHL, S], F32R)

            # PSUM accumulators for all projection phases (6 banks; psc has 2)
            pacc_cm = tc.tile_pool(name="pacc", bufs=6, space="PSUM")
            pacc = pacc_cm.__enter__()

            # ---------- phases 1-3: QT (bias+rope), latT, KT (rope)
            with (
                tc.tile_pool(name="xp", bufs=1) as xp,
                tc.tile_pool(name="ws1", bufs=6) as ws1,
                tc.tile_pool(name="rope1", bufs=3) as rp1,
            ):
                xT_sb = xp.tile([P, KT_H, S], F32R)
                nc.sync.dma_start(bq_sb, bqd)

                def proj_og(w_dram, rhs_sb, n_kt, og, wtag, load_x=False,
                            n_oi=2, wpool=None):
                    """One out group: accumulate n_oi x NQC psum tiles."""
                    wpool = wpool or ws1
                    ps = {}
                    for oi in range(n_oi):
                        for ntc in range(NQC):
                            ps[(oi, ntc)] = pacc.tile(
                                [P, QCW], F32, tag="acc", name="acc"
                            )
                    for kt in range(n_kt):
                        if load_x:
                            # stream xT chunk just-in-time (og 0 only)
                            nc.sync.dma_start(
                                xT_sb[:, kt, :],
                                xT[kt * P:(kt + 1) * P, :].bitcast(F32R),
                            )
                        wt = wpool.tile([P, n_oi * P], F32R, tag=wtag, name="wt")
                        nc.sync.dma_start(
                            wt,
                            w_dram[kt * P:(kt + 1) * P,
                                   og * n_oi * P:(og + 1) * n_oi * P
                                   ].bitcast(F32R),
                        )
                        for oi in range(n_oi):
                            for ntc in range(NQC):
                                nc.tensor.matmul(
                                    ps[(oi, ntc)],
                                    lhsT=wt[:, oi * P:(oi + 1) * P],
                                    rhs=rhs_sb[:, kt, ntc * QCW:(ntc + 1) * QCW],
                                    start=(kt == 0),
                                    stop=(kt == n_kt - 1),
                                )
                    return ps

                # QT: bias + rope per head. The first group covers 4 heads
                # (8 accumulators: 6 from pacc + 2 borrowed from the idle
                # attention scores pool) so the PE stays fed for the whole
                # ~29us xT-streaming window.
                def qt_rope(h, ps_oi_ntc):
                    raw = rp1.tile([P, S], F32, tag="raw", name="raw")
                    for ntc in range(NQC):
                        nc.scalar.add(
                            raw[:, ntc * QCW:(ntc + 1) * QCW],
                            ps_oi_ntc[ntc],
                            bq_sb[:, h:h + 1],
                        )
                    _rope(nc, rp1, raw, qT_sb[:, h, :], cosT_sb, sinTe_sb,
                          dve_sin=(h % 2 == 0))

                # mega-group: heads 0-3
                ps = {}
                for oi in range(4):
                    for ntc in range(NQC):
                        pool = pacc if (oi, ntc) < (3, 0) else psc
                        tag = "acc" if pool is pacc else "sc"
                        ps[(oi, ntc)] = pool.tile(
                            [P, QCW], F32, tag=tag, name="acc"
                        )
                for kt in range(KT_H):
                    nc.sync.dma_start(
                        xT_sb[:, kt, :],
                        xT[kt * P:(kt + 1) * P, :].bitcast(F32R),
                    )
                    wt = ws1.tile([P, 4 * P], F32R, tag="wq4", name="wt")
                    nc.sync.dma_start(
                        wt, wq[kt * P:(kt + 1) * P, 0:4 * P].bitcast(F32R)
                    )
                    for oi in range(4):
                        for ntc in range(NQC):
                            nc.tensor.matmul(
                                ps[(oi, ntc)],
                                lhsT=wt[:, oi * P:(oi + 1) * P],
                                rhs=xT_sb[:, kt, ntc * QCW:(ntc + 1) * QCW],
                                start=(kt == 0),
                                stop=(kt == KT_H - 1),
                            )
                nc.sync.dma_start(cosT_sb, cosTd)
                nc.sync.dma_start(sinTe_sb, sinTed)
                for oi in range(4):
                    qt_rope(oi, {ntc: ps[(oi, ntc)] for ntc in range(NQC)})

                # heads 4-7 in pair groups
                for og in range(2, 4):
                    ps = proj_og(wq, xT_sb, KT_H, og, "wq")
                    for oi in range(2):
                        h = og * 2 + oi
                        qt_rope(h, {ntc: ps[(oi, ntc)] for ntc in range(NQC)})

                # latT (og 0 borrows the idle scores-PSUM banks so it can
                # start before QT og3's accumulators drain)
                for og in range(2):
                    if og == 0:
                        ps = {}
                        for oi in range(2):
                            for ntc in range(NQC):
                                pool = pacc if oi == 0 else psc
                                tag = "acc" if pool is pacc else "sc"
                                ps[(oi, ntc)] = pool.tile(
                                    [P, QCW], F32, tag=tag, name="acc"
                                )
                        for kt in range(KT_H):
                            wt = ws1.tile([P, 2 * P], F32R, tag="wd", name="wt")
                            nc.sync.dma_start(
                                wt,
                                wdown[kt * P:(kt + 1) * P, 0:2 * P].bitcast(F32R),
                            )
                            for oi in range(2):
                                for ntc in range(NQC):
                                    nc.tensor.matmul(
                                        ps[(oi, ntc)],
                                        lhsT=wt[:, oi * P:(oi + 1) * P],
                                        rhs=xT_sb[:, kt,
                                                  ntc * QCW:(ntc + 1) * QCW],
                                        start=(kt == 0),
                                        stop=(kt == KT_H - 1),
                                    )
                    else:
                        ps = proj_og(wdown, xT_sb, KT_H, og, "wd")
                    for oi in range(2):
                        for ntc in range(NQC):
                            nc.scalar.copy(
                                latT_sb[:, og * 2 + oi, ntc * QCW:(ntc + 1) * QCW],
                                ps[(oi, ntc)],
                            )

            # xT / ws1 / rope1 freed here

            # ---------- phase 4: V natural [S, HL] (reuses xT's SBUF zone)
            vpool_cm = tc.tile_pool(name="vres", bufs=1)
            vpool = vpool_cm.__enter__()
            v_sb = vpool.tile([P, NHL, HL], F32R)  # [s%128, s//128, hl]
            with tc.tile_pool(name="ws2", bufs=6) as ws2:
                for hlc in range(2):
                    for sg in range(2):
                        ps = {}
                        for si in range(4):
                            ps[si] = pacc.tile([P, QCW], F32, tag="acc", name="acc")
                        for kt in range(KT_L):
                            wt = ws2.tile([P, QCW], F32R, tag="wv", name="wt")
                            nc.sync.dma_start(
                                wt,
                                wvup[kt * P:(kt + 1) * P,
                                     hlc * QCW:(hlc + 1) * QCW].bitcast(F32R),
                            )
                            for si in range(4):
                                st = sg * 4 + si
                                nc.tensor.matmul(
                                    ps[si],
                                    lhsT=latT_sb[:, kt, st * P:(st + 1) * P],
                                    rhs=wt,
                                    start=(kt == 0),
                                    stop=(kt == KT_L - 1),
                                )
                        for si in range(4):
                            st = sg * 4 + si
                            nc.scalar.copy(
                                v_sb[:, st, hlc * QCW:(hlc + 1) * QCW], ps[si]
                            )

            # ---------- phase 5: KT (rope per head, contract latT over LAT)
            with (
                tc.tile_pool(name="ws3", bufs=4) as ws1,
                tc.tile_pool(name="rope3", bufs=3) as rp1,
            ):
                # KT: rope per head (contract latT over LAT)
                for og in range(4):
                    ps = proj_og(wkup, latT_sb, KT_L, og, "wk")
                    if og == 0:
                        # attention-phase constants join the DMA queue here
                        nc.sync.dma_start(mask_sb, maskTd)
                        nc.sync.dma_start(ones_col, onescd.bitcast(F32R))
                        nc.sync.dma_start(ones_k, oneskd.bitcast(F32R))
                    for oi in range(2):
                        h = og * 2 + oi
                        raw = rp1.tile([P, S], F32, tag="raw", name="raw")
                        for ntc in range(NQC):
                            nc.scalar.copy(
                                raw[:, ntc * QCW:(ntc + 1) * QCW], ps[(oi, ntc)]
                            )
                        _rope(nc, rp1, raw, kT_sb[:, h, :], cosT_sb, sinTe_sb,
                              dve_sin=(oi == 0), dma_shift=True)

            pacc_cm.__exit__(None, None, None)  # free PSUM for attention pools

            # ---------- attention + output projection
            with (
                tc.tile_pool(name="ctxp", bufs=1) as ctxp,
                tc.tile_pool(name="exl", bufs=8) as exl,
                tc.tile_pool(name="small", bufs=3) as small,
                tc.tile_pool(name="pctx", bufs=2, space="PSUM") as pctx,
                tc.tile_pool(name="psum1", bufs=2, space="PSUM") as psum1,
                tc.tile_pool(name="pbcpo", bufs=2, space="PSUM") as pbcpo,
                tc.tile_pool(name="wos", bufs=4) as wos,
                tc.tile_pool(name="outsb", bufs=3) as outsb,
            ):
                ctxT_sb = ctxp.tile([P, NHL, S], F32R)

                def finalize(fin):
                    ctx_f, sums_f, h_f, qc_f = fin
                    srow = small.tile([1, QCW], F32R, tag="srow", name="srow")
                    nc.any.tensor_copy(srow, sums_f)
                    bc = pbcpo.tile([P, QCW], F32, tag="bcpo", name="bc")
                    nc.tensor.matmul(
                        bc, lhsT=ones_col, rhs=srow, start=True, stop=True
                    )
                    rec = small.tile([P, QCW], F32, tag="rec", name="rec")
                    nc.vector.reciprocal(out=rec, in_=bc)
                    nc.vector.tensor_mul(
                        ctxT_sb[:, h_f, qc_f * QCW:(qc_f + 1) * QCW], ctx_f, rec
                    )

                pending = None
                for qc in range(NQC):
                    for h in range(NHL):
                        nkt = 4 * qc + 4  # k-tiles covering causal range
                        ctx = pctx.tile([P, QCW], F32, tag="ctx")
                        sums = psum1.tile([1, QCW], F32, tag="sums")

                        def block_geom(kt):
                            """(lo, w, mask_ap): sliced q-range for causal."""
                            off = kt - 4 * qc
                            if off < 0:
                                return 0, QCW, None
                            if off < 3:
                                # triangle sits in the first 128 sliced cols
                                return 128 * off, QCW - 128 * off, \
                                    mask_sb[:, 0:P]
                            # off == 3: keep moving width >= 256 for fp32r
                            return 256, 256, mask_sb[:, P:3 * P]

                        def emit_sc(kt):
                            lo, w, mk = block_geom(kt)
                            sc = psc.tile([P, QCW], F32, tag="sc", name="sc")
                            nc.tensor.matmul(
                                sc[:, :w],
                                lhsT=kT_sb[:, h, kt * P:(kt + 1) * P],
                                rhs=qT_sb[:, h,
                                          qc * QCW + lo:qc * QCW + lo + w],
                                start=True,
                                stop=True,
                            )
                            ex = exl.tile([P, QCW], F32R, tag="ex", name="ex")
                            nc.scalar.activation(
                                out=ex[:, :w], in_=sc[:, :w],
                                func=mybir.ActivationFunctionType.Exp,
                                scale=SC_SCALE,
                            )
                            if mk is not None:  # causal zeroing of the triangle
                                mw = mk.shape[-1]
                                eng = nc.vector if (kt % 2) else nc.gpsimd
                                eng.tensor_mul(ex[:, :mw], ex[:, :mw], mk)
                            return ex

                        def emit_pv(kt, ex):
                            lo, w, _ = block_geom(kt)
                            nc.tensor.matmul(
                                ctx[:, lo:lo + w],
                                lhsT=v_sb[:, kt, h * P:(h + 1) * P],
                                rhs=ex[:, :w],
                                start=(kt == 0),
                                stop=(kt == nkt - 1),
                            )
                            nc.tensor.matmul(
                                sums[:, lo:lo + w],
                                lhsT=ones_k,
                                rhs=ex[:, :w],
                                start=(kt == 0),
                                stop=(kt == nkt - 1),
                            )

                        # software-pipelined emission: sc(kt+1) before pv(kt)
                        exs = {0: emit_sc(0)}
                        for kt in range(nkt):
                            if kt + 1 < nkt:
                                exs[kt + 1] = emit_sc(kt + 1)
                            emit_pv(kt, exs.pop(kt))
                        # normalize the PREVIOUS head so its srow copy has a
                        # whole head of PE work to hide behind
                        if pending is not None:
                            finalize(pending)
                        pending = (ctx, sums, h, qc)
                finalize(pending)

                # out-projection: outT[o, s] = sum_hl Wo[hl, o] * ctxT[hl, s]
                for ot in range(HID // P):
                    wt = wos.tile([P, NHL, P], F32R, tag="wo", name="wt")
                    nc.sync.dma_start(
                        wt,
                        wo[:, ot * P:(ot + 1) * P].rearrange(
                            "(kt p) o -> p kt o", p=P
                        ).bitcast(F32R),
                    )
                    for qc in range(NQC):
                        po = pbcpo.tile([P, QCW], F32, tag="bcpo", name="po")
                        for kt in range(NHL):
                            nc.tensor.matmul(
                                po,
                                lhsT=wt[:, kt, :],
                                rhs=ctxT_sb[:, kt, qc * QCW:(qc + 1) * QCW],
                                start=(kt == 0),
                                stop=(kt == NHL - 1),
                            )
                        osb = outsb.tile([P, QCW], F32, tag="osb")
                        nc.any.tensor_copy(osb, po)
                        nc.sync.dma_start(
                            outT[ot * P:(ot + 1) * P, qc * QCW:(qc + 1) * QCW],
                            osb,
                        )
            vpool_cm.__exit__(None, None, None)
    nc.compile()
    return nc


# ---------------- host side ----------------

def _host_consts():
    inv_freq = 1.0 / (10000.0 ** (np.arange(0, HD, 2, dtype=np.float64) / HD))
    t = np.arange(S, dtype=np.float64)
    freqs = t[:, None] * inv_freq[None, :]            # [S, 64]
    emb = np.concatenate([freqs, freqs], axis=-1)     # [S, 128]
    cosT = np.cos(emb).T.astype(np.float32).copy()    # [128, S]
    sinT = np.sin(emb).T.astype(np.float32)
    sinTe = sinT.copy()
    sinTe[:64] *= -1.0                                # sign of rotate_half folded in
    sinTe = np.ascontiguousarray(sinTe.astype(np.float32))

    ii = np.arange(P)[:, None]
    tri = (np.arange(P)[None, :] - ii >= 0).astype(np.float32)       # [128,128]
    maskb = np.concatenate([np.zeros((P, P), np.float32), tri], axis=1)
    maskT = np.ascontiguousarray(np.concatenate([tri, maskb], axis=1))  # [128,384]
    return cosT, sinTe, maskT


_CACHE = {}


def _get_built():
    if "nc" not in _CACHE:
        _CACHE["nc"] = build_bass()
        _CACHE["consts"] = _host_consts()
    return _CACHE["nc"], _CACHE["consts"]


def make_in_maps(x, Wq, bq, Wdown, Wk_up, Wv_up, Wo):
    cosT, sinTe, maskT = _get_built()[1]
    in_maps = []
    for c in range(N_CORES):
        b, hg = c // 2, c % 2
        sl = slice(hg * HL, (hg + 1) * HL)
        in_maps.append({
            "xT": np.ascontiguousarray(x[b].T),
            "wq": np.ascontiguousarray(Wq[:, sl]),
            "wdown": np.ascontiguousarray(Wdown),
            "wkup": np.ascontiguousarray(Wk_up[:, sl]),
            "wvup": np.ascontiguousarray(Wv_up[:, sl]),
            "wo": np.ascontiguousarray(Wo[sl, :]),
            "bq": np.ascontiguousarray(bq[sl].reshape(NHL, P).T),
            "cosT": cosT,
            "sinTe": sinTe,
            "maskT": maskT,
            "ones_c": np.ones((1, P), np.float32),
            "ones_r": np.ones((P, 1), np.float32),
        })
    return in_maps


def gather_out(results, bo):
    out = np.empty((B, S, HID), dtype=np.float32)
    for b in range(B):
        acc = results[2 * b]["outT"] + results[2 * b + 1]["outT"]  # [HID, S]
        out[b] = acc.T + bo[None, :]
    return out


def kernel(x, Wq, bq, Wdown, Wk_up, Wv_up, Wo, bo):
    x = np.asarray(x, dtype=np.float32)
    Wq = np.asarray(Wq, dtype=np.float32)
    bq = np.asarray(bq, dtype=np.float32)
    Wdown = np.asarray(Wdown, dtype=np.float32)
    Wk_up = np.asarray(Wk_up, dtype=np.float32)
    Wv_up = np.asarray(Wv_up, dtype=np.float32)
    Wo = np.asarray(Wo, dtype=np.float32)
    bo = np.asarray(bo, dtype=np.float32)

    nc, _ = _get_built()
    in_maps = make_in_maps(x, Wq, bq, Wdown, Wk_up, Wv_up, Wo)
    res = run_bass_kernel_spmd(nc, in_maps, core_ids=list(range(N_CORES)))
    return gather_out(res.results, bo)



# revision 2
# speedup vs baseline: 2.3094x; 2.3094x over previous
"""Trainium2 Bass kernel: MultiHeadLatentAttention (v2, bf16 + occupancy).

Problem (hardcoded): B=4, S=1024, HID=2048, NH=16 heads of HD=128, LAT=512,
fp32 in/out, causal attention with RoPE, latent-compressed K/V (MLA).

Sharding over 8 NeuronCores: core c = (batch b = c//2, head-group hg = c%2).
Each core handles one batch element and 8 heads (local width HL=1024).

v2 design (vs v1):
  * bf16 operands everywhere on-device (fp32 PSUM accumulation). Halves DMA
    and SBUF, 2x DVE element ops; PE rate is identical to fp32r per the cost
    model but without the ap>=256 constraint, enabling minimal causal widths.
  * Causal mask applied as a PE "triangle add" into the scores PSUM
    (matmul lhsT=I, rhs=triM accumulated before stop) - removes all DVE/Pool
    mask multiplies from the sc->exp->pv chain.
  * All weights except Wq resident in SBUF (loaded once per iteration on the
    scalar engine's DMA queue; xT/Wq stream on sync's queue; outT stores on
    vector's queue).
  * PSUM: 4 pools x 2 banks (sc/ctx/nm/po). Projection phases borrow the
    same slots via tags, ordered so the hardware loop pipelines across
    iterations (next iteration's first accumulators alias banks that free
    early in the previous iteration).
  * Emission interleaves V with K^T projection groups, and the qc=0 output
    projection with qc=1 attention, so the PE never drains while serial
    chains (exp, rope, finalize) run on Act/DVE/Pool.
"""

import os

if "axon" not in os.environ.get("JAX_PLATFORMS", ""):
    os.environ["JAX_PLATFORMS"] = "axon"

import numpy as np
import ml_dtypes

import concourse.bacc as bacc
import concourse.mybir as mybir
import concourse.tile as tile
from concourse.bass_utils import run_bass_kernel_spmd

# ---- problem dims (hardcoded per contest rules)
B, S, HID, NH, LAT = 4, 1024, 2048, 16, 512
HD = 128
NHL = NH // 2          # heads per core = 8
HL = NHL * HD          # local head width = 1024
P = 128
KT_H = HID // P        # 16
KT_L = LAT // P        # 4
QCW = 512              # q-chunk width (PSUM bank)
NQC = S // QCW         # 2
NOT = HID // P         # 16 out-proj row tiles
SC_SCALE = float(1.0 / np.sqrt(HD))
MASK_NEG = -9984.0     # bf16-exact large negative; exp(scale*(-9984)) == 0

F32 = mybir.dt.float32
BF16 = mybir.dt.bfloat16

N_CORES = 8


def build_bass(loop_iters=None):
    nc = bacc.Bacc("TRN2", target_bir_lowering=False, debug=False, num_devices=8)

    xT = nc.dram_tensor("xT", [HID, S], BF16, kind="ExternalInput")[:]
    wq = nc.dram_tensor("wq", [HID, HL], BF16, kind="ExternalInput")[:]
    wdown = nc.dram_tensor("wdown", [HID, LAT], BF16, kind="ExternalInput")[:]
    wkup = nc.dram_tensor("wkup", [LAT, HL], BF16, kind="ExternalInput")[:]
    wvup = nc.dram_tensor("wvup", [LAT, HL], BF16, kind="ExternalInput")[:]
    wopk = nc.dram_tensor("wopk", [P, NOT, NHL, P], BF16, kind="ExternalInput")[:]
    bqd = nc.dram_tensor("bq", [P, NHL], F32, kind="ExternalInput")[:]
    cosTd = nc.dram_tensor("cosT", [P, S], BF16, kind="ExternalInput")[:]
    sinTed = nc.dram_tensor("sinTe", [P, S], BF16, kind="ExternalInput")[:]
    triMd = nc.dram_tensor("triM", [P, P], BF16, kind="ExternalInput")[:]
    eyed = nc.dram_tensor("eyeP", [P, P], BF16, kind="ExternalInput")[:]
    oneskd = nc.dram_tensor("ones_k", [P, 1], BF16, kind="ExternalInput")[:]
    onescd = nc.dram_tensor("ones_c", [1, P], BF16, kind="ExternalInput")[:]
    outT = nc.dram_tensor("outT", [HID, S], F32, kind="ExternalOutput")[:]

    import contextlib

    with tile.TileContext(nc) as tc, contextlib.ExitStack() as _les:
        _les.enter_context(
            nc.allow_low_precision(reason="bf16 kernel; fp32 PSUM accum")
        )
        if loop_iters is not None:
            _les.enter_context(tc.For_i(0, loop_iters, 1))
        with (
            tc.tile_pool(name="consts", bufs=1) as consts,
            tc.tile_pool(name="resident", bufs=1) as resident,
            tc.tile_pool(name="wres", bufs=1) as wres,
            tc.tile_pool(name="ws", bufs=4) as ws,
            tc.tile_pool(name="rp", bufs=3) as rp,
            tc.tile_pool(name="exl", bufs=6) as exl,
            tc.tile_pool(name="small", bufs=2) as small,
            tc.tile_pool(name="outsb", bufs=3) as outsb,
            tc.tile_pool(name="psc", bufs=2, space="PSUM") as psc,
            tc.tile_pool(name="pctx", bufs=2, space="PSUM") as pctx,
            tc.tile_pool(name="pnorm", bufs=2, space="PSUM") as pnorm,
            tc.tile_pool(name="po", bufs=2, space="PSUM") as po,
        ):
            cosT_sb = consts.tile([P, S], BF16)
            sinTe_sb = consts.tile([P, S], BF16)
            tri_sb = consts.tile([P, P], BF16)
            eye_sb = consts.tile([P, P], BF16)
            bq_sb = consts.tile([P, NHL], F32)
            ones_k = consts.tile([P, 1], BF16)
            ones_c = consts.tile([1, P], BF16)

            xT_sb = resident.tile([P, KT_H, S], BF16)
            latT_sb = resident.tile([P, KT_L, S], BF16)
            qT_sb = resident.tile([P, NHL, S], BF16)
            kT_sb = resident.tile([P, NHL, S], BF16)
            v_sb = resident.tile([P, NHL, HL], BF16)
            ctxT_sb = resident.tile([P, NHL, S], BF16)

            wdown_sb = wres.tile([P, KT_H, LAT], BF16)
            wkup_sb = wres.tile([P, KT_L, HL], BF16)
            wvup_sb = wres.tile([P, KT_L, HL], BF16)
            wo_sb = wres.tile([P, NOT, NHL, P], BF16)

            # ---- resident/const loads on the scalar engine's DMA queue
            nc.scalar.dma_start(bq_sb, bqd)
            nc.scalar.dma_start(cosT_sb, cosTd)
            nc.scalar.dma_start(sinTe_sb, sinTed)
            nc.scalar.dma_start(tri_sb, triMd)
            nc.scalar.dma_start(eye_sb, eyed)
            nc.scalar.dma_start(ones_k, oneskd)
            nc.scalar.dma_start(ones_c, onescd)
            for kt in range(KT_H):
                nc.scalar.dma_start(
                    wdown_sb[:, kt, :], wdown[kt * P:(kt + 1) * P, :]
                )
            for kt in range(KT_L):
                nc.scalar.dma_start(
                    wkup_sb[:, kt, :], wkup[kt * P:(kt + 1) * P, :]
                )
                nc.scalar.dma_start(
                    wvup_sb[:, kt, :], wvup[kt * P:(kt + 1) * P, :]
                )
            for oc in range(8):  # wo in 8 chunks to share the DMA fairly
                nc.scalar.dma_start(
                    wo_sb[:, 2 * oc:2 * oc + 2, :, :],
                    wopk[:, 2 * oc:2 * oc + 2, :, :],
                )

            # ---------------- helpers ----------------
            def acc_alloc(sets):
                """Allocate accumulators [(pool, tag), ...] -> list of tiles."""
                return [
                    pool.tile([P, QCW], F32, tag=tag, name="acc")
                    for pool, tag in sets
                ]

            SET4A = [(pctx, "ctx"), (pctx, "ctx"), (pnorm, "nm"), (pnorm, "nm")]
            SET4B = [(po, "po"), (po, "po"), (psc, "sc"), (psc, "sc")]

            def _rope(raw, out_ap, sin_pool=False):
                """out = raw*cosT + shift64(raw)*sinTe (sin sign pre-folded)."""
                sh = rp.tile([P, S], BF16, tag="sh", name="sh")
                nc.scalar.dma_start(sh[0:64, :], raw[64:128, :])
                nc.scalar.dma_start(sh[64:128, :], raw[0:64, :])
                nc.vector.tensor_mul(out_ap, raw, cosT_sb)
                if sin_pool:
                    nc.gpsimd.tensor_mul(sh, sh, sinTe_sb)
                else:
                    nc.vector.tensor_mul(sh, sh, sinTe_sb)
                nc.vector.tensor_add(out_ap, out_ap, sh)

            # ---------------- phase 1: Q (rope) + latT ----------------
            # Q head-groups (2,3,3); og0 avoids the po banks so the next
            # loop iteration can start while the previous outT stores drain.
            def q_og(h0, nh, sets, load_x=False):
                accs = acc_alloc(sets)  # nh*2 tiles: [(oi,ntc)] row-major
                for kt in range(KT_H):
                    if load_x:
                        if kt == 0:  # split first tile: earlier first matmul
                            nc.sync.dma_start(
                                xT_sb[:, 0, 0:QCW], xT[0:P, 0:QCW]
                            )
                            nc.sync.dma_start(
                                xT_sb[:, 0, QCW:S], xT[0:P, QCW:S]
                            )
                        else:
                            nc.sync.dma_start(
                                xT_sb[:, kt, :], xT[kt * P:(kt + 1) * P, :]
                            )
                    wt = ws.tile([P, nh * P], BF16, tag="wt", name="wt")
                    nc.sync.dma_start(
                        wt, wq[kt * P:(kt + 1) * P, h0 * P:(h0 + nh) * P]
                    )
                    for ntc in range(NQC):
                        for oi in range(nh):
                            nc.tensor.matmul(
                                accs[oi * NQC + ntc],
                                lhsT=wt[:, oi * P:(oi + 1) * P],
                                rhs=xT_sb[:, kt, ntc * QCW:(ntc + 1) * QCW],
                                start=(kt == 0),
                                stop=(kt == KT_H - 1),
                            )
                # all bias-adds first (frees accumulators early), then ropes
                raws = []
                for oi in range(nh):
                    h = h0 + oi
                    raw = rp.tile([P, S], BF16, tag="raw", name="raw")
                    for ntc in range(NQC):
                        nc.scalar.add(
                            raw[:, ntc * QCW:(ntc + 1) * QCW],
                            accs[oi * NQC + ntc],
                            bq_sb[:, h:h + 1],
                        )
                    raws.append((h, raw))
                for h, raw in raws:
                    _rope(raw, qT_sb[:, h, :], sin_pool=(h % 3 == 2))

            q_og(0, 2, SET4A, load_x=True)
            q_og(2, 3, SET4A + [(po, "po"), (po, "po")])
            q_og(5, 3, SET4A + [(po, "po"), (po, "po")])

            # latT: 2 groups of 2 LAT-blocks
            for og in range(2):
                sets = SET4A if og == 0 else SET4B
                accs = acc_alloc(sets)  # [(blk, ntc)]
                for kt in range(KT_H):
                    for ntc in range(NQC):
                        for blk in range(2):
                            nc.tensor.matmul(
                                accs[blk * NQC + ntc],
                                lhsT=wdown_sb[:, kt,
                                              (og * 2 + blk) * P:
                                              (og * 2 + blk + 1) * P],
                                rhs=xT_sb[:, kt, ntc * QCW:(ntc + 1) * QCW],
                                start=(kt == 0),
                                stop=(kt == KT_H - 1),
                            )
                for blk in range(2):
                    for ntc in range(NQC):
                        nc.scalar.copy(
                            latT_sb[:, og * 2 + blk,
                                    ntc * QCW:(ntc + 1) * QCW],
                            accs[blk * NQC + ntc],
                        )

            # ---------------- phase 2/3: V and K^T as po-pool subgroups ---
            # Each subgroup holds the po pool's 2 banks for 8 matmuls, then
            # drains via Act copies; attention qc0 owns psc/pctx/pnorm and is
            # fed interleaved with the remaining subgroups via a filler queue.
            def v_sub(hlc, sg, pair):
                sts = (sg * 4 + 2 * pair, sg * 4 + 2 * pair + 1)
                state = {}

                def mk_mm(kt, i):
                    def f():
                        if "a" not in state:
                            state["a"] = [
                                po.tile([P, QCW], F32, tag="po", name="va")
                                for _ in range(2)
                            ]
                        nc.tensor.matmul(
                            state["a"][i],
                            lhsT=latT_sb[:, kt, sts[i] * P:(sts[i] + 1) * P],
                            rhs=wvup_sb[:, kt, hlc * QCW:(hlc + 1) * QCW],
                            start=(kt == 0),
                            stop=(kt == KT_L - 1),
                        )
                    return f

                units = [mk_mm(kt, i) for kt in range(KT_L) for i in range(2)]

                def drain():
                    for i, st in enumerate(sts):
                        nc.scalar.copy(
                            v_sb[:, st, hlc * QCW:(hlc + 1) * QCW],
                            state["a"][i],
                        )

                units.append(drain)
                return units

            def kt_sub(h):
                state = {}

                def mk_mm(kt, ntc):
                    def f():
                        if "a" not in state:
                            state["a"] = [
                                po.tile([P, QCW], F32, tag="po", name="ka")
                                for _ in range(2)
                            ]
                        nc.tensor.matmul(
                            state["a"][ntc],
                            lhsT=wkup_sb[:, kt, h * P:(h + 1) * P],
                            rhs=latT_sb[:, kt, ntc * QCW:(ntc + 1) * QCW],
                            start=(kt == 0),
                            stop=(kt == KT_L - 1),
                        )
                    return f

                units = [mk_mm(kt, ntc) for kt in range(KT_L)
                         for ntc in range(NQC)]

                def drain():
                    raw = rp.tile([P, S], BF16, tag="raw", name="raw")
                    for ntc in range(NQC):
                        nc.scalar.copy(
                            raw[:, ntc * QCW:(ntc + 1) * QCW], state["a"][ntc]
                        )
                    _rope(raw, kT_sb[:, h, :], sin_pool=(h % 3 == 2))

                units.append(drain)
                return units

            # subgroups needed before attention qc0 can run heads 0-3 / 4-7
            pre = (v_sub(0, 0, 0) + v_sub(0, 0, 1) + kt_sub(0) + kt_sub(1)
                   + v_sub(1, 0, 0) + v_sub(1, 0, 1) + kt_sub(2) + kt_sub(3))
            for u in pre:
                u()

            fill_units = (kt_sub(4) + kt_sub(5) + kt_sub(6) + kt_sub(7)
                          + v_sub(0, 1, 0) + v_sub(0, 1, 1)
                          + v_sub(1, 1, 0) + v_sub(1, 1, 1))
            fq = {"u": fill_units}

            def fill_take(n):
                for _ in range(n):
                    if fq["u"]:
                        fq["u"].pop(0)()

            # ---------------- attention + output projection --------------
            def make_outproj(qc):
                """Returns (emit_n, flush) for the qc output projection."""
                state = {"ot": 0, "k": 0, "pt": None}

                def emit(n):
                    for _ in range(n):
                        ot, k = state["ot"], state["k"]
                        if ot >= NOT:
                            return
                        if k == 0:
                            state["pt"] = po.tile(
                                [P, QCW], F32, tag="po", name="pot"
                            )
                        nc.tensor.matmul(
                            state["pt"],
                            lhsT=wo_sb[:, ot, k, :],
                            rhs=ctxT_sb[:, k, qc * QCW:(qc + 1) * QCW],
                            start=(k == 0),
                            stop=(k == NHL - 1),
                        )
                        if k == NHL - 1:
                            osb = outsb.tile([P, QCW], F32, tag="osb",
                                             name="osb")
                            if ot % 2:
                                nc.vector.tensor_copy(osb, state["pt"])
                            else:
                                nc.scalar.copy(osb, state["pt"])
                            nc.gpsimd.dma_start(
                                outT[ot * P:(ot + 1) * P,
                                     qc * QCW:(qc + 1) * QCW],
                                osb,
                            )
                            state["ot"], state["k"] = ot + 1, 0
                        else:
                            state["k"] = k + 1

                def flush():
                    while state["ot"] < NOT:
                        emit(1)

                return emit, flush

            def attn_head(qc, h, filler=None):
                nkt = 4 * qc + 4
                ctx = pctx.tile([P, QCW], F32, tag="ctx", name="ctx")
                sums = pnorm.tile([1, QCW], F32, tag="nm", name="sums")

                def geom(kt):
                    off = kt - 4 * qc
                    if off < 0:
                        return 0, QCW, False
                    return P * off, QCW - P * off, True

                def emit_sc(kt, bi):
                    lo, w, tri = geom(kt)
                    sc = psc.tile([P, QCW], F32, tag="sc", name="sct")
                    nc.tensor.matmul(
                        sc[:, :w],
                        lhsT=kT_sb[:, h, kt * P:(kt + 1) * P],
                        rhs=qT_sb[:, h, qc * QCW + lo:qc * QCW + lo + w],
                        start=True,
                        stop=not tri,
                    )
                    if tri:
                        nc.tensor.matmul(
                            sc[:, 0:P], lhsT=eye_sb, rhs=tri_sb,
                            start=False, stop=True,
                        )
                    ex = exl.tile([P, QCW], BF16, tag="ex", name="ex")
                    nc.scalar.activation(
                        out=ex[:, :w], in_=sc[:, :w],
                        func=mybir.ActivationFunctionType.Exp,
                        scale=SC_SCALE,
                    )
                    return ex

                def emit_pv(kt, ex):
                    lo, w, _ = geom(kt)
                    nc.tensor.matmul(
                        ctx[:, lo:lo + w],
                        lhsT=v_sb[:, kt, h * P:(h + 1) * P],
                        rhs=ex[:, :w],
                        start=(kt == 0),
                        stop=(kt == nkt - 1),
                    )
                    nc.tensor.matmul(
                        sums[:, lo:lo + w],
                        lhsT=ones_k,
                        rhs=ex[:, :w],
                        start=(kt == 0),
                        stop=(kt == nkt - 1),
                    )

                exs = {0: emit_sc(0, 0)}
                for kt in range(nkt):
                    if kt + 1 < nkt:
                        exs[kt + 1] = emit_sc(kt + 1, kt + 1)
                    emit_pv(kt, exs.pop(kt))
                    if filler is not None:
                        filler(2)

                # immediate finalize: srow -> bcast -> recip -> scale ctxT
                srow = small.tile([1, QCW], BF16, tag="srow", name="srow")
                nc.vector.tensor_copy(srow, sums)
                bc = pnorm.tile([P, QCW], F32, tag="nm", name="bc")
                nc.tensor.matmul(bc, lhsT=ones_c, rhs=srow,
                                 start=True, stop=True)
                rec = small.tile([P, QCW], BF16, tag="rec", name="rec")
                nc.vector.reciprocal(out=rec, in_=bc)
                nc.vector.tensor_mul(
                    ctxT_sb[:, h, qc * QCW:(qc + 1) * QCW], ctx, rec
                )

            for h in range(NHL):
                attn_head(0, h, filler=fill_take)
            fill_take(len(fill_units))

            emit0, flush0 = make_outproj(0)
            for h in range(NHL):
                attn_head(1, h, filler=emit0)
            flush0()

            _, flush1 = make_outproj(1)
            flush1()

    nc.compile()
    return nc


# ---------------- host side ----------------

def _host_consts():
    inv_freq = 1.0 / (10000.0 ** (np.arange(0, HD, 2, dtype=np.float64) / HD))
    t = np.arange(S, dtype=np.float64)
    freqs = t[:, None] * inv_freq[None, :]            # [S, 64]
    emb = np.concatenate([freqs, freqs], axis=-1)     # [S, 128]
    cosT = np.cos(emb).T.astype(ml_dtypes.bfloat16)   # [128, S]
    sinT = np.sin(emb).T.astype(np.float32)
    sinTe = sinT.copy()
    sinTe[:64] *= -1.0                                # rotate_half sign folded
    sinTe = np.ascontiguousarray(sinTe.astype(ml_dtypes.bfloat16))
    cosT = np.ascontiguousarray(cosT)

    a = np.arange(P)
    triM = np.where(a[None, :] < a[:, None], np.float32(MASK_NEG),
                    np.float32(0.0)).astype(ml_dtypes.bfloat16)  # [k, q]
    eyeP = np.eye(P, dtype=ml_dtypes.bfloat16)
    return cosT, sinTe, np.ascontiguousarray(triM), np.ascontiguousarray(eyeP)


_CACHE = {}


def _get_built():
    if "nc" not in _CACHE:
        _CACHE["nc"] = build_bass()
        _CACHE["consts"] = _host_consts()
    return _CACHE["nc"], _CACHE["consts"]


def make_in_maps(x, Wq, bq, Wdown, Wk_up, Wv_up, Wo):
    cosT, sinTe, triM, eyeP = _get_built()[1]
    bf = ml_dtypes.bfloat16
    in_maps = []
    for c in range(N_CORES):
        b, hg = c // 2, c % 2
        sl = slice(hg * HL, (hg + 1) * HL)
        wo_loc = np.asarray(Wo)[sl, :]                       # [HL, HID]
        wopk = np.ascontiguousarray(
            wo_loc.reshape(NHL, P, NOT, P).transpose(1, 2, 0, 3)
        ).astype(bf)                                         # [p, ot, kt, o]
        in_maps.append({
            "xT": np.ascontiguousarray(np.asarray(x)[b].T).astype(bf),
            "wq": np.ascontiguousarray(np.asarray(Wq)[:, sl]).astype(bf),
            "wdown": np.ascontiguousarray(np.asarray(Wdown)).astype(bf),
            "wkup": np.ascontiguousarray(np.asarray(Wk_up)[:, sl]).astype(bf),
            "wvup": np.ascontiguousarray(np.asarray(Wv_up)[:, sl]).astype(bf),
            "wopk": wopk,
            "bq": np.ascontiguousarray(
                np.asarray(bq)[sl].reshape(NHL, P).T
            ).astype(np.float32),
            "cosT": cosT,
            "sinTe": sinTe,
            "triM": triM,
            "eyeP": eyeP,
            "ones_k": np.ones((P, 1), bf),
            "ones_c": np.ones((1, P), bf),
        })
    return in_maps


def gather_out(results, bo):
    out = np.empty((B, S, HID), dtype=np.float32)
    bo = np.asarray(bo, dtype=np.float32)
    for b in range(B):
        acc = results[2 * b]["outT"] + results[2 * b + 1]["outT"]  # [HID, S]
        out[b] = acc.T + bo[None, :]
    return out


def kernel(x, Wq, bq, Wdown, Wk_up, Wv_up, Wo, bo):
    nc, _ = _get_built()
    in_maps = make_in_maps(x, Wq, bq, Wdown, Wk_up, Wv_up, Wo)
    res = run_bass_kernel_spmd(nc, in_maps, core_ids=list(range(N_CORES)))
    return gather_out(res.results, bo)


# revision 8
# speedup vs baseline: 2.3739x; 1.0279x over previous
"""Trainium2 Bass kernel: MultiHeadLatentAttention (v2, bf16 + occupancy).

Problem (hardcoded): B=4, S=1024, HID=2048, NH=16 heads of HD=128, LAT=512,
fp32 in/out, causal attention with RoPE, latent-compressed K/V (MLA).

Sharding over 8 NeuronCores: core c = (batch b = c//2, head-group hg = c%2).
Each core handles one batch element and 8 heads (local width HL=1024).

v2 design (vs v1):
  * bf16 operands everywhere on-device (fp32 PSUM accumulation). Halves DMA
    and SBUF, 2x DVE element ops; PE rate is identical to fp32r per the cost
    model but without the ap>=256 constraint, enabling minimal causal widths.
  * Causal mask applied as a PE "triangle add" into the scores PSUM
    (matmul lhsT=I, rhs=triM accumulated before stop) - removes all DVE/Pool
    mask multiplies from the sc->exp->pv chain.
  * All weights except Wq resident in SBUF (loaded once per iteration on the
    scalar engine's DMA queue; xT/Wq stream on sync's queue; outT stores on
    vector's queue).
  * PSUM: 4 pools x 2 banks (sc/ctx/nm/po). Projection phases borrow the
    same slots via tags, ordered so the hardware loop pipelines across
    iterations (next iteration's first accumulators alias banks that free
    early in the previous iteration).
  * Emission interleaves V with K^T projection groups, and the qc=0 output
    projection with qc=1 attention, so the PE never drains while serial
    chains (exp, rope, finalize) run on Act/DVE/Pool.
"""

import os

if "axon" not in os.environ.get("JAX_PLATFORMS", ""):
    os.environ["JAX_PLATFORMS"] = "axon"

import numpy as np
import ml_dtypes

import concourse.bacc as bacc
import concourse.mybir as mybir
import concourse.tile as tile
from concourse.bass_utils import run_bass_kernel_spmd

# ---- problem dims (hardcoded per contest rules)
B, S, HID, NH, LAT = 4, 1024, 2048, 16, 512
HD = 128
NHL = NH // 2          # heads per core = 8
HL = NHL * HD          # local head width = 1024
P = 128
KT_H = HID // P        # 16
KT_L = LAT // P        # 4
QCW = 512              # q-chunk width (PSUM bank)
NQC = S // QCW         # 2
NOT = HID // P         # 16 out-proj row tiles
SC_SCALE = float(1.0 / np.sqrt(HD))
MASK_NEG = -9984.0     # bf16-exact large negative; exp(scale*(-9984)) == 0

F32 = mybir.dt.float32
BF16 = mybir.dt.bfloat16

N_CORES = 8


def build_bass(loop_iters=None, body_repeats=None):
    nc = bacc.Bacc("TRN2", target_bir_lowering=False, debug=False, num_devices=8)

    xT = nc.dram_tensor("xT", [HID, S], BF16, kind="ExternalInput")[:]
    wq = nc.dram_tensor("wq", [HID, HL], BF16, kind="ExternalInput")[:]
    wdown = nc.dram_tensor("wdown", [HID, LAT], BF16, kind="ExternalInput")[:]
    wkup = nc.dram_tensor("wkup", [LAT, HL], BF16, kind="ExternalInput")[:]
    wvup = nc.dram_tensor("wvup", [LAT, HL], BF16, kind="ExternalInput")[:]
    wopk = nc.dram_tensor("wopk", [P, NOT, NHL, P], BF16, kind="ExternalInput")[:]
    bqd = nc.dram_tensor("bq", [P, NHL], F32, kind="ExternalInput")[:]
    cosTd = nc.dram_tensor("cosT", [P, S], BF16, kind="ExternalInput")[:]
    sinTed = nc.dram_tensor("sinTe", [P, S], BF16, kind="ExternalInput")[:]
    triMd = nc.dram_tensor("triM", [P, P], BF16, kind="ExternalInput")[:]
    eyed = nc.dram_tensor("eyeP", [P, P], BF16, kind="ExternalInput")[:]
    oneskd = nc.dram_tensor("ones_k", [P, 1], BF16, kind="ExternalInput")[:]
    onescd = nc.dram_tensor("ones_c", [1, P], BF16, kind="ExternalInput")[:]
    outT = nc.dram_tensor("outT", [HID, S], F32, kind="ExternalOutput")[:]

    import contextlib

    if body_repeats is None:
        # unroll the body inside For_i: amortizes the ~59us loop back-edge
        # and lets consecutive bodies pipeline. Cap emitted bodies at 12.
        body_repeats = 1
        if loop_iters is not None:
            for r in range(min(12, loop_iters), 0, -1):
                if loop_iters % r == 0:
                    body_repeats = r
                    break
    with tile.TileContext(nc) as tc, contextlib.ExitStack() as _les:
        _les.enter_context(
            nc.allow_low_precision(reason="bf16 kernel; fp32 PSUM accum")
        )
        if loop_iters is not None and loop_iters // body_repeats > 1:
            _les.enter_context(tc.For_i(0, loop_iters // body_repeats, 1))
        with (
            tc.tile_pool(name="consts", bufs=1) as consts,
            tc.tile_pool(name="resident", bufs=1) as resident,
            tc.tile_pool(name="wres", bufs=1) as wres,
            tc.tile_pool(name="ws", bufs=2) as ws,
            tc.tile_pool(name="rp", bufs=3) as rp,
            tc.tile_pool(name="exl", bufs=6) as exl,
            tc.tile_pool(name="small", bufs=2) as small,
            tc.tile_pool(name="outsb", bufs=2) as outsb,
            tc.tile_pool(name="psc", bufs=2, space="PSUM") as psc,
            tc.tile_pool(name="pctx", bufs=2, space="PSUM") as pctx,
            tc.tile_pool(name="pnorm", bufs=2, space="PSUM") as pnorm,
            tc.tile_pool(name="po", bufs=2, space="PSUM") as po,
        ):
          def emit_body():
            cosT_sb = consts.tile([P, S], BF16)
            sinTe_sb = consts.tile([P, S], BF16)
            tri_sb = consts.tile([P, P], BF16)
            eye_sb = consts.tile([P, P], BF16)
            bq_sb = consts.tile([P, NHL], F32)
            ones_k = consts.tile([P, 1], BF16)
            ones_c = consts.tile([1, P], BF16)

            xT_sb = resident.tile([P, KT_H, S], BF16)
            latT_sb = resident.tile([P, KT_L, S], BF16)
            qT_sb = resident.tile([P, NHL, S], BF16)
            kT_sb = resident.tile([P, NHL, S], BF16)
            v_sb = resident.tile([P, NHL, HL], BF16)
            ctxT_sb = resident.tile([P, NHL, S], BF16)

            wdown_sb = wres.tile([P, KT_H, LAT], BF16)
            wkup_sb = wres.tile([P, KT_L, HL], BF16)
            wvup_sb = wres.tile([P, KT_L, HL], BF16)
            wo_sb = wres.tile([P, NOT, NHL, P], BF16)

            # ---- resident/const loads on the scalar engine's DMA queue
            # (batched: few large DMAs -- per-DMA overhead dominates on HW)
            nc.scalar.dma_start(bq_sb, bqd)
            nc.scalar.dma_start(cosT_sb, cosTd)
            nc.scalar.dma_start(sinTe_sb, sinTed)
            nc.scalar.dma_start(tri_sb, triMd)
            nc.scalar.dma_start(eye_sb, eyed)
            nc.scalar.dma_start(ones_k, oneskd)
            nc.scalar.dma_start(ones_c, onescd)
            for g in range(4):
                nc.scalar.dma_start(
                    wdown_sb[:, 4 * g:4 * g + 4, :],
                    wdown[4 * g * P:(4 * g + 4) * P, :].rearrange(
                        "(k p) w -> p k w", p=P
                    ),
                )
            nc.scalar.dma_start(
                wkup_sb, wkup.rearrange("(k p) w -> p k w", p=P)
            )
            nc.scalar.dma_start(
                wvup_sb, wvup.rearrange("(k p) w -> p k w", p=P)
            )
            for oc in range(4):  # wo in 4 chunks to share the DMA fairly
                nc.scalar.dma_start(
                    wo_sb[:, 4 * oc:4 * oc + 4, :, :],
                    wopk[:, 4 * oc:4 * oc + 4, :, :],
                )

            # ---------------- helpers ----------------
            def acc_alloc(sets):
                """Allocate accumulators [(pool, tag), ...] -> list of tiles."""
                return [
                    pool.tile([P, QCW], F32, tag=tag, name="acc")
                    for pool, tag in sets
                ]

            SET4A = [(pctx, "ctx"), (pctx, "ctx"), (pnorm, "nm"), (pnorm, "nm")]
            SET4B = [(po, "po"), (po, "po"), (psc, "sc"), (psc, "sc")]

            def _rope(raw, out_ap, sin_pool=False):
                """out = raw*cosT + shift64(raw)*sinTe (sin sign pre-folded).
                Shift done with engine copies (partition-offset copies are
                legal; offset TensorTensor is not), no DMA."""
                sh = rp.tile([P, S], BF16, tag="sh", name="sh", bufs=2)
                nc.vector.tensor_copy(sh[0:64, :], raw[64:128, :])
                nc.gpsimd.tensor_copy(sh[64:128, :], raw[0:64, :])
                nc.vector.tensor_mul(out_ap, raw, cosT_sb)
                if sin_pool:
                    nc.gpsimd.tensor_mul(sh, sh, sinTe_sb)
                else:
                    nc.vector.tensor_mul(sh, sh, sinTe_sb)
                nc.vector.tensor_add(out_ap, out_ap, sh)

            # ---------------- phase 1: Q (rope) + latT ----------------
            # Q head-groups (2,3,3); og0 avoids the po banks so the next
            # loop iteration can start while the previous outT stores drain.
            def q_og(h0, nh, sets, load_x=False):
                accs = acc_alloc(sets)  # nh*2 tiles: [(oi,ntc)] row-major
                wt = None
                for kt in range(KT_H):
                    if load_x:
                        # batched x stream: kt0 split for cold-start ramp,
                        # then 4-kt chunks
                        if kt == 0:
                            nc.sync.dma_start(
                                xT_sb[:, 0, 0:QCW], xT[0:P, 0:QCW]
                            )
                            nc.sync.dma_start(
                                xT_sb[:, 0, QCW:S], xT[0:P, QCW:S]
                            )
                            nc.sync.dma_start(
                                xT_sb[:, 1:4, :],
                                xT[P:4 * P, :].rearrange(
                                    "(k p) s -> p k s", p=P
                                ),
                            )
                        elif kt % 4 == 0:
                            nc.sync.dma_start(
                                xT_sb[:, kt:kt + 4, :],
                                xT[kt * P:(kt + 4) * P, :].rearrange(
                                    "(k p) s -> p k s", p=P
                                ),
                            )
                    if kt % 4 == 0:  # 4-kt weight chunk
                        wt = ws.tile([P, 4, nh * P], BF16, tag="wt",
                                     name="wt")
                        nc.sync.dma_start(
                            wt,
                            wq[kt * P:(kt + 4) * P,
                               h0 * P:(h0 + nh) * P].rearrange(
                                "(k p) w -> p k w", p=P
                            ),
                        )
                    for oi in range(nh):
                        for ntc in range(NQC):
                            nc.tensor.matmul(
                                accs[oi * NQC + ntc],
                                lhsT=wt[:, kt % 4, oi * P:(oi + 1) * P],
                                rhs=xT_sb[:, kt, ntc * QCW:(ntc + 1) * QCW],
                                start=(kt == 0),
                                stop=(kt == KT_H - 1),
                            )
                # all bias-adds first (frees accumulators early), then ropes
                raws = []
                for oi in range(nh):
                    h = h0 + oi
                    raw = rp.tile([P, S], BF16, tag="raw", name="raw")
                    for ntc in range(NQC):
                        nc.scalar.add(
                            raw[:, ntc * QCW:(ntc + 1) * QCW],
                            accs[oi * NQC + ntc],
                            bq_sb[:, h:h + 1],
                        )
                    raws.append((h, raw))
                for h, raw in raws:
                    _rope(raw, qT_sb[:, h, :], sin_pool=(h % 2 == 1))

            q_og(0, 2, SET4A, load_x=True)
            q_og(2, 3, SET4A + [(po, "po"), (po, "po")])
            q_og(5, 3, SET4A + [(po, "po"), (po, "po")])

            # latT: 2 groups of 2 LAT-blocks
            for og in range(2):
                sets = SET4A if og == 0 else SET4B
                accs = acc_alloc(sets)  # [(blk, ntc)]
                for kt in range(KT_H):
                    for ntc in range(NQC):
                        for blk in range(2):
                            nc.tensor.matmul(
                                accs[blk * NQC + ntc],
                                lhsT=wdown_sb[:, kt,
                                              (og * 2 + blk) * P:
                                              (og * 2 + blk + 1) * P],
                                rhs=xT_sb[:, kt, ntc * QCW:(ntc + 1) * QCW],
                                start=(kt == 0),
                                stop=(kt == KT_H - 1),
                            )
                for blk in range(2):
                    for ntc in range(NQC):
                        nc.scalar.copy(
                            latT_sb[:, og * 2 + blk,
                                    ntc * QCW:(ntc + 1) * QCW],
                            accs[blk * NQC + ntc],
                        )

            # ---------------- phase 2/3: V and K^T as po-pool subgroups ---
            # Each subgroup holds the po pool's 2 banks for 8 matmuls, then
            # drains via Act copies; attention qc0 owns psc/pctx/pnorm and is
            # fed interleaved with the remaining subgroups via a filler queue.
            def v_sub(hlc, sg, pair):
                sts = (sg * 4 + 2 * pair, sg * 4 + 2 * pair + 1)
                state = {}

                def mk_mm(kt, i):
                    def f():
                        if "a" not in state:
                            state["a"] = [
                                po.tile([P, QCW], F32, tag="po", name="va")
                                for _ in range(2)
                            ]
                        nc.tensor.matmul(
                            state["a"][i],
                            lhsT=latT_sb[:, kt, sts[i] * P:(sts[i] + 1) * P],
                            rhs=wvup_sb[:, kt, hlc * QCW:(hlc + 1) * QCW],
                            start=(kt == 0),
                            stop=(kt == KT_L - 1),
                        )
                    return f

                units = [mk_mm(kt, i) for kt in range(KT_L) for i in range(2)]

                def drain():
                    for i, st in enumerate(sts):
                        nc.scalar.copy(
                            v_sb[:, st, hlc * QCW:(hlc + 1) * QCW],
                            state["a"][i],
                        )

                units.append(drain)
                return units

            def kt_sub(h):
                state = {}

                def mk_mm(kt, ntc):
                    def f():
                        if "a" not in state:
                            state["a"] = [
                                po.tile([P, QCW], F32, tag="po", name="ka")
                                for _ in range(2)
                            ]
                        nc.tensor.matmul(
                            state["a"][ntc],
                            lhsT=wkup_sb[:, kt, h * P:(h + 1) * P],
                            rhs=latT_sb[:, kt, ntc * QCW:(ntc + 1) * QCW],
                            start=(kt == 0),
                            stop=(kt == KT_L - 1),
                        )
                    return f

                units = [mk_mm(kt, ntc) for kt in range(KT_L)
                         for ntc in range(NQC)]

                def drain():
                    raw = rp.tile([P, S], BF16, tag="raw", name="raw")
                    for ntc in range(NQC):
                        nc.scalar.copy(
                            raw[:, ntc * QCW:(ntc + 1) * QCW], state["a"][ntc]
                        )
                    _rope(raw, kT_sb[:, h, :], sin_pool=(h % 2 == 1))

                units.append(drain)
                return units

            # subgroups needed before attention qc0 can run heads 0-3 / 4-7
            pre = (v_sub(0, 0, 0) + v_sub(0, 0, 1) + kt_sub(0) + kt_sub(1)
                   + v_sub(1, 0, 0) + v_sub(1, 0, 1) + kt_sub(2) + kt_sub(3))
            for u in pre:
                u()

            fill_units = (kt_sub(4) + kt_sub(5) + kt_sub(6) + kt_sub(7)
                          + v_sub(0, 1, 0) + v_sub(0, 1, 1)
                          + v_sub(1, 1, 0) + v_sub(1, 1, 1))
            fq = {"u": fill_units}

            def fill_take(n):
                for _ in range(n):
                    if fq["u"]:
                        fq["u"].pop(0)()

            # ---------------- attention + output projection --------------
            def make_outproj(qc):
                """Returns (emit_n, flush) for the qc output projection."""
                state = {"ot": 0, "k": 0, "pt": None}

                def emit(n):
                    for _ in range(n):
                        ot, k = state["ot"], state["k"]
                        if ot >= NOT:
                            return
                        if k == 0:
                            state["pt"] = po.tile(
                                [P, QCW], F32, tag="po", name="pot"
                            )
                            if ot % 2 == 0:
                                state["osb"] = outsb.tile(
                                    [P, 2, QCW], F32, tag="osb", name="osb"
                                )
                        nc.tensor.matmul(
                            state["pt"],
                            lhsT=wo_sb[:, ot, k, :],
                            rhs=ctxT_sb[:, k, qc * QCW:(qc + 1) * QCW],
                            start=(k == 0),
                            stop=(k == NHL - 1),
                        )
                        if k == NHL - 1:
                            osb = state["osb"]
                            if ot % 2:
                                nc.vector.tensor_copy(
                                    osb[:, 1, :], state["pt"]
                                )
                                # one store per ot pair; gpsimd queue so
                                # loads (SP) never queue behind stores
                                nc.gpsimd.dma_start(
                                    outT[(ot - 1) * P:(ot + 1) * P,
                                         qc * QCW:(qc + 1) * QCW].rearrange(
                                        "(k p) s -> p k s", p=P
                                    ),
                                    osb,
                                )
                            else:
                                nc.scalar.copy(osb[:, 0, :], state["pt"])
                            state["ot"], state["k"] = ot + 1, 0
                        else:
                            state["k"] = k + 1

                def flush():
                    while state["ot"] < NOT:
                        emit(1)

                return emit, flush

            def attn_head(qc, h, filler=None):
                nkt = 4 * qc + 4
                ctx = pctx.tile([P, QCW], F32, tag="ctx", name="ctx")
                sums = pnorm.tile([1, QCW], F32, tag="nm", name="sums")

                def geom(kt):
                    off = kt - 4 * qc
                    if off < 0:
                        return 0, QCW, False
                    return P * off, QCW - P * off, True

                def emit_sc(kt, bi):
                    lo, w, tri = geom(kt)
                    sc = psc.tile([P, QCW], F32, tag="sc", name="sct")
                    nc.tensor.matmul(
                        sc[:, :w],
                        lhsT=kT_sb[:, h, kt * P:(kt + 1) * P],
                        rhs=qT_sb[:, h, qc * QCW + lo:qc * QCW + lo + w],
                        start=True,
                        stop=not tri,
                    )
                    if tri:
                        nc.tensor.matmul(
                            sc[:, 0:P], lhsT=eye_sb, rhs=tri_sb,
                            start=False, stop=True,
                        )
                    ex = exl.tile([P, QCW], BF16, tag="ex", name="ex")
                    nc.scalar.activation(
                        out=ex[:, :w], in_=sc[:, :w],
                        func=mybir.ActivationFunctionType.Exp,
                        scale=SC_SCALE,
                    )
                    return ex

                def emit_pv(kt, ex):
                    lo, w, _ = geom(kt)
                    nc.tensor.matmul(
                        ctx[:, lo:lo + w],
                        lhsT=v_sb[:, kt, h * P:(h + 1) * P],
                        rhs=ex[:, :w],
                        start=(kt == 0),
                        stop=(kt == nkt - 1),
                    )
                    nc.tensor.matmul(
                        sums[:, lo:lo + w],
                        lhsT=ones_k,
                        rhs=ex[:, :w],
                        start=(kt == 0),
                        stop=(kt == nkt - 1),
                    )

                exs = {0: emit_sc(0, 0)}
                for kt in range(nkt):
                    if kt + 1 < nkt:
                        exs[kt + 1] = emit_sc(kt + 1, kt + 1)
                    emit_pv(kt, exs.pop(kt))
                    if filler is not None:
                        filler(2)

                # immediate finalize: srow -> bcast -> recip -> scale ctxT
                srow = small.tile([1, QCW], BF16, tag="srow", name="srow")
                nc.vector.tensor_copy(srow, sums)
                bc = pnorm.tile([P, QCW], F32, tag="nm", name="bc")
                nc.tensor.matmul(bc, lhsT=ones_c, rhs=srow,
                                 start=True, stop=True)
                rec = small.tile([P, QCW], BF16, tag="rec", name="rec")
                nc.vector.reciprocal(out=rec, in_=bc)
                nc.vector.tensor_mul(
                    ctxT_sb[:, h, qc * QCW:(qc + 1) * QCW], ctx, rec
                )

            for h in range(NHL):
                attn_head(0, h, filler=fill_take)
            fill_take(len(fill_units))

            emit0, flush0 = make_outproj(0)
            for h in range(NHL):
                attn_head(1, h, filler=emit0)
            flush0()

            _, flush1 = make_outproj(1)
            flush1()

          for _rep in range(body_repeats):
            emit_body()

    nc.compile()
    return nc


# ---------------- host side ----------------

def _host_consts():
    inv_freq = 1.0 / (10000.0 ** (np.arange(0, HD, 2, dtype=np.float64) / HD))
    t = np.arange(S, dtype=np.float64)
    freqs = t[:, None] * inv_freq[None, :]            # [S, 64]
    emb = np.concatenate([freqs, freqs], axis=-1)     # [S, 128]
    cosT = np.cos(emb).T.astype(ml_dtypes.bfloat16)   # [128, S]
    sinT = np.sin(emb).T.astype(np.float32)
    sinTe = sinT.copy()
    sinTe[:64] *= -1.0                                # rotate_half sign folded
    sinTe = np.ascontiguousarray(sinTe.astype(ml_dtypes.bfloat16))
    cosT = np.ascontiguousarray(cosT)

    a = np.arange(P)
    triM = np.where(a[None, :] < a[:, None], np.float32(MASK_NEG),
                    np.float32(0.0)).astype(ml_dtypes.bfloat16)  # [k, q]
    eyeP = np.eye(P, dtype=ml_dtypes.bfloat16)
    return cosT, sinTe, np.ascontiguousarray(triM), np.ascontiguousarray(eyeP)


_CACHE = {}


def _get_built():
    if "nc" not in _CACHE:
        _CACHE["nc"] = build_bass()
        _CACHE["consts"] = _host_consts()
    return _CACHE["nc"], _CACHE["consts"]


def make_in_maps(x, Wq, bq, Wdown, Wk_up, Wv_up, Wo):
    cosT, sinTe, triM, eyeP = _get_built()[1]
    bf = ml_dtypes.bfloat16
    in_maps = []
    for c in range(N_CORES):
        b, hg = c // 2, c % 2
        sl = slice(hg * HL, (hg + 1) * HL)
        wo_loc = np.asarray(Wo)[sl, :]                       # [HL, HID]
        wopk = np.ascontiguousarray(
            wo_loc.reshape(NHL, P, NOT, P).transpose(1, 2, 0, 3)
        ).astype(bf)                                         # [p, ot, kt, o]
        in_maps.append({
            "xT": np.ascontiguousarray(np.asarray(x)[b].T).astype(bf),
            "wq": np.ascontiguousarray(np.asarray(Wq)[:, sl]).astype(bf),
            "wdown": np.ascontiguousarray(np.asarray(Wdown)).astype(bf),
            "wkup": np.ascontiguousarray(np.asarray(Wk_up)[:, sl]).astype(bf),
            "wvup": np.ascontiguousarray(np.asarray(Wv_up)[:, sl]).astype(bf),
            "wopk": wopk,
            "bq": np.ascontiguousarray(
                np.asarray(bq)[sl].reshape(NHL, P).T
            ).astype(np.float32),
            "cosT": cosT,
            "sinTe": sinTe,
            "triM": triM,
            "eyeP": eyeP,
            "ones_k": np.ones((P, 1), bf),
            "ones_c": np.ones((1, P), bf),
        })
    return in_maps


def gather_out(results, bo):
    out = np.empty((B, S, HID), dtype=np.float32)
    bo = np.asarray(bo, dtype=np.float32)
    for b in range(B):
        acc = results[2 * b]["outT"] + results[2 * b + 1]["outT"]  # [HID, S]
        out[b] = acc.T + bo[None, :]
    return out


def kernel(x, Wq, bq, Wdown, Wk_up, Wv_up, Wo, bo):
    nc, _ = _get_built()
    in_maps = make_in_maps(x, Wq, bq, Wdown, Wk_up, Wv_up, Wo)
    res = run_bass_kernel_spmd(nc, in_maps, core_ids=list(range(N_CORES)))
    return gather_out(res.results, bo)
